# revision 36
# baseline (speedup 1.0000x reference)
"""CKConv (nn_CKConv_85950885527678) Trainium2 Bass kernel.

Strategy: data-parallel over batch (8 batches -> 8 NeuronCores). The tiny
SIREN kernel network runs on the host; the generated conv kernel is
replicated to every core (per the sharding hint). The bias add also runs
on the host (it would otherwise sit on the device critical path).

Per core the causal conv out[o,t] = sum_{i,l>=1} K[o,i,l] * xp[i,t+l]
(xp = x left-padded with T zeros) is computed with full-width 128x128
matmuls via a *diagonal* decomposition: the causal boundary t+l >= 2048 is
a diagonal in (time, tap) space, so four time-tiles spaced S apart share
one moving-operand slice, each receiving a tap block shifted by S taps.
Every matmul therefore uses all 128 stationary columns (4 tiles x 32 out
channels) and a full-depth contraction.

Default implementation (CK_IMPL=fp8): split-precision float8_e4m3 with
DoubleRow perf mode (K = 256 = 8 taps x 32 in channels, two 4-tap k-tiles
packed per partition; the k-tile's +4-tap shift is folded into a 3D moving
AP with a 4-column offset). out = W8*X8 + W8*X8r + W8r*X8 accumulates in
one f32 psum; the dropped W8r*X8r term is ~0.4% relative, giving overall
~0.002 rel err (better than plain bf16 at ~0.004). A bf16 fallback
(CK_IMPL=bf16, K=128) is kept.

The stationary tables ship compact (each tap block once) and are expanded
4x on-chip by the otherwise-idle DVE/Act/Pool engines (bit-preserving
uint16-bitcast copies, overlapping-window APs), overlapped with PE
compute; a short warmup keeps the PE p-state ramped while the first DMA
pieces land.
"""

import os
import numpy as np

B, C_IN, C_OUT, T, D = 8, 32, 32, 2048, 32
L = T + 1
U0 = 1534
XPW = 2565
S = int(os.environ.get("CK_S", "32"))   # time-tile size
NPASS = T // (4 * S)
NJ = 512                # stationary col-blocks
NBR = 512               # compact table blocks (real, b descending)
N_CORES = 8

_cache = {}


# ---------------------------------------------------------------- host prep

def _siren_kernel(pos_rel, w1, b1, w2, b2, w3, b3):
    p = pos_rel.reshape(1, L).astype(np.float32)
    h = np.sin(w1.astype(np.float32) @ p + b1[:, None].astype(np.float32))
    h = np.sin(w2.astype(np.float32) @ h + b2[:, None].astype(np.float32))
    k = w3.astype(np.float32) @ h + b3[:, None].astype(np.float32)
    return k.astype(np.float32)


def _build_wcr(k):
    """Compact reversed table (real blocks only): col (br-96)*32 + o holds
    K[o, i, 4b+1+dd] at partition dd*32+i, with b = 607-br, br in [96,608)."""
    kk = k.reshape(C_OUT, C_IN, L)[:, :, 1:]
    arr = kk.reshape(C_OUT, C_IN, NJ, 4).transpose(3, 1, 2, 0)  # [dd,i,b,o]
    w = arr.reshape(128, NJ, C_OUT)[:, ::-1, :]                 # b descending
    return np.ascontiguousarray(w.reshape(128, NJ * C_OUT)).astype(np.float32)


def _build_xp(x):
    xpad = np.zeros((B, C_IN, 2 * T + 8), np.float32)
    xpad[:, :, T : 2 * T] = x
    XP = np.empty((B, 128, XPW), np.float32)
    for dd in range(4):
        XP[:, dd * 32 : (dd + 1) * 32, :] = xpad[:, :, U0 + dd : U0 + dd + XPW]
    return XP


def _build_fp8_tables(k):
    """Split-precision fp8 tables.

    Compact stationary layout (kt pairs adjacent, 8-tap blocks descending):
    wc[dd*32+i, m, o] = W[o, i, 4*block4(m) + dd] with
    block4(m) = 510 - m + 2*(m & 1)  (m in [0, 512)).
    """
    import ml_dtypes

    F8 = ml_dtypes.float8_e4m3fn
    kk = k.reshape(C_OUT, C_IN, L)[:, :, 1:]          # [o, i, tap 0..2047]
    w8 = kk.astype(F8)
    w8r = (kk - w8.astype(np.float32)).astype(F8)

    m = np.arange(512)
    block4 = 510 - m + 2 * (m & 1)
    tap0 = 4 * block4

    def compact(w):
        # -> [dd*32+i, m, o]
        taps = w.reshape(C_OUT, C_IN, 512, 4)          # [o, i, b, dd]
        arr = taps[:, :, block4, :]                     # [o, i, m, dd]
        return np.ascontiguousarray(
            arr.transpose(3, 1, 2, 0).reshape(128, 512 * C_OUT)
        )

    return compact(w8), compact(w8r)


def _build_xp8(x):
    import ml_dtypes

    F8 = ml_dtypes.float8_e4m3fn
    xpad = np.zeros((B, C_IN, 2 * T + 8), np.float32)
    xpad[:, :, T : 2 * T] = x
    x8 = xpad.astype(F8)
    x8r = (xpad - x8.astype(np.float32)).astype(F8)
    XP8 = np.empty((B, 128, XPW), F8)
    XP8r = np.empty((B, 128, XPW), F8)
    for dd in range(4):
        XP8[:, dd * 32 : (dd + 1) * 32, :] = x8[:, :, U0 + dd : U0 + dd + XPW]
        XP8r[:, dd * 32 : (dd + 1) * 32, :] = x8r[:, :, U0 + dd : U0 + dd + XPW]
    return XP8, XP8r


# ------------------------------------------------------- tile drain patch

def _patch_tile_drain():
    """This walrus build rejects >2 sync waits on a CTRL (Drain) instruction;
    spread the TileContext exit waits over single-wait NOPs instead."""
    from concourse.tile import TileContext
    from concourse.vector_clock import ScopedClock, VectorClock

    if getattr(TileContext, "_ck_drain_patched", False):
        return

    def _drain_and_barrier(self, tick_clock, wait_clock):
        light = os.environ.get("CK_LIGHT_DRAIN", "2")
        gc = tick_clock.global_clock
        n = len(gc)
        if light != "2":
            for p in range(n):
                if gc[p] <= 0:
                    continue
                vec = [gc[q] if q == p else 0 for q in range(n)]
                nop = self.nc.sync.nop(nofuse=True, hint=f"split_drain_wait_p{p}")
                wait_clock.add_sem_waits(
                    nop.ins, ScopedClock({None: VectorClock(vec)})
                )
        self.nc.sync.drain()
        if light != "2":
            self.nc.all_engine_barrier()
        assert self.sems is not None
        popped = self.nc._tile_sem_poison_stack.pop()
        assert popped is self._sem_poison
        if os.environ.get("CK_LIGHT_DRAIN", "2") == "0":
            self.nc.clear_and_free_semaphores(list(self.sems.allocated().values()))
            self.nc.all_engine_barrier()
        else:
            pass  # leave sems allocated; program ends here anyway

    TileContext._drain_and_barrier = _drain_and_barrier
    TileContext._ck_drain_patched = True


WAIT_LIMIT = 1  # this walrus build encodes at most 2 sync waits per instruction


def _split_excess_waits(nc, limit=WAIT_LIMIT):
    """Hoist excess sem waits onto same-engine NOPs placed just before the
    instruction — in-order engine queues make this semantically identical."""
    import concourse.mybir as mybir

    n_split = 0
    for f in nc.m.functions:
        for bb in f.blocks:
            new_insts = []
            changed = False
            for inst in bb.instructions:
                si = inst.sync_info
                waits = list(si.on_wait) if si is not None and si.on_wait else []
                if len(waits) > limit:
                    extra, keep = waits[:-limit], waits[-limit:]
                    for i in range(0, len(extra), limit):
                        n_split += 1
                        new_insts.append(
                            mybir.InstNoOp(
                                name=f"I-ckwsplit-{n_split}",
                                engine=inst.engine,
                                ins=[],
                                outs=[],
                                sync_info=mybir.SyncInfo(
                                    on_wait=extra[i : i + limit], on_update=[]
                                ),
                            )
                        )
                    inst.sync_info = mybir.SyncInfo(
                        on_wait=keep, on_update=list(si.on_update) if si.on_update else []
                    )
                    changed = True
                new_insts.append(inst)
            if changed:
                bb.instructions = new_insts
    return n_split


# ------------------------------------------------------------ device kernel

def _build_nc():
    import concourse.bass as bass
    import concourse.mybir as mybir
    from concourse.tile import TileContext

    _patch_tile_drain()
    f32 = mybir.dt.float32
    bf16 = mybir.dt.bfloat16

    nc = bass.Bass()
    xp_d = nc.declare_dram_parameter("xp", [128, XPW], bf16, isOutput=False)
    wcr_d = nc.declare_dram_parameter("wcr", [128, NJ * 32], bf16, isOutput=False)
    out_d = nc.declare_dram_parameter("out", [128, NPASS * S], f32, isOutput=True)

    max_passes = int(os.environ.get("CK_MAX_PASSES", str(NPASS)))
    n_warmup = int(os.environ.get("CK_WARMUP", "240"))

    with TileContext(nc) as tc:
        with (
            tc.tile_pool(name="const", bufs=1) as const,
            tc.tile_pool(name="work", bufs=1) as work,
            tc.tile_pool(name="acc_psum", bufs=7, space="PSUM") as acc_psum,
            tc.tile_pool(name="warm_psum", bufs=1, space="PSUM") as warm_psum,
        ):
            xp_sb = const.tile([128, XPW], bf16)
            wcr_sb = const.tile([128, NBR, 32], bf16)
            exp_sb = const.tile([128, NJ, 128], bf16)
            out_sb = work.tile([128, NPASS * S], f32)

            # DMA order = first-use order. Pass 0 only needs xp cols
            # >= 515-S and the first 32 compact blocks; xp cols [0, 384)
            # are never read.
            def _wcr_piece(lo, hi):
                nc.sync.dma_start(
                    wcr_sb[:, lo:hi, :],
                    wcr_d[:, lo * 32 : hi * 32].rearrange(
                        "p (a b) -> p a b", b=32
                    ),
                )

            def _xp_piece(lo, hi):
                nc.sync.dma_start(xp_sb[:, lo:hi], xp_d[:, lo:hi])

            xp_layout = os.environ.get("CK_XP", "V3")
            if xp_layout == "V2":
                _wcr_piece(0, 32)
                _xp_piece(480, XPW)
                _wcr_piece(32, 64)
                _wcr_piece(64, 96)
                for k in range(3, 16):
                    _wcr_piece(32 * k, 32 * k + 32)
            else:
                _wcr_piece(0, 32)
                _xp_piece(480, 896)
                _wcr_piece(32, 64)
                _wcr_piece(64, 96)
                _xp_piece(896, 1664)
                _wcr_piece(96, 128)
                _wcr_piece(128, 160)
                _xp_piece(1664, XPW)
                for k in range(5, 16):
                    _wcr_piece(32 * k, 32 * k + 32)

            # p-state warmup: keep the PE busy with trivial matmuls while
            # the first stationary chunks arrive, so real matmuls start at
            # full clock. Feed from a memset scratch so warmup needs no DMA.
            if n_warmup:
                scratch = const.tile([128, 16], bf16)
                nc.vector.memset(scratch[:, :], 1.0)
                warm_ps = warm_psum.tile([16, 16], f32)
                for _ in range(n_warmup):
                    nc.tensor.matmul(
                        warm_ps[:, :], scratch[:, :], scratch[:, :],
                        start=True, stop=True,
                    )

            # On-chip 4x expansion: exp[:, j, 32q:+32] =
            # wcr[:, j + (S/4)q - 3S/4, :] (zero where the block index is
            # out of range, i.e. j < 3S/4 - (S/4)q), issued in 16 steps of
            # 32 j each, each step emitted just before the first pass that
            # reads it. q=0..2 on DVE, q=3 on the Activation engine.
            zlen = [max(0, 3 * S // 4 - (S // 4) * q) for q in range(4)]
            for q in range(4):
                if zlen[q]:
                    # zero corners on the idle Pool engine, off DVE's path
                    nc.gpsimd.memset(
                        exp_sb[:, 0 : zlen[q], 32 * q : 32 * q + 32], 0.0
                    )

            from concourse.ap import AP

            def _issue_step(kstep):
                j0 = 32 * kstep
                if j0 >= zlen[0]:
                    # q1..q3 in one fused DVE copy (overlapping-window src
                    # AP [part, j, q, col]); q0 alternates Act/Pool
                    base = wcr_sb[:, :, :]
                    srcw = AP(
                        base.tensor,
                        base.offset + (j0 - (S // 2)) * 32,
                        [list(base.ap[0]), [32, 32], [(S // 4) * 32, 3], [1, 32]],
                    )
                    dst = exp_sb[:, j0 : j0 + 32, 32:128].rearrange(
                        "p a (b c) -> p a b c", b=3
                    )
                    nc.vector.tensor_copy(dst, srcw)
                    q0src = wcr_sb[:, j0 - 3 * S // 4 : j0 + 32 - 3 * S // 4, :]
                    q0dst = exp_sb[:, j0 : j0 + 32, 0:32]
                    if kstep % 2 == 0:
                        nc.scalar.activation(
                            q0dst, q0src,
                            mybir.ActivationFunctionType.Copy, bias=0.0,
                        )
                    else:
                        nc.gpsimd.tensor_copy(q0dst, q0src)
                else:
                    # first step: all chunks on DVE (small, zero-clipped) so
                    # the slower engines stay off the startup critical path
                    for q in range(3, -1, -1):
                        lo = max(j0, zlen[q])       # first non-zero j
                        if lo >= j0 + 32:
                            continue
                        src_lo = lo + (S // 4) * q - 3 * S // 4
                        nc.vector.tensor_copy(
                            exp_sb[:, lo : j0 + 32, 32 * q : 32 * q + 32],
                            wcr_sb[:, src_lo : src_lo + (j0 + 32 - lo), :],
                        )

            steps_issued = 0
            for p in range(max_passes):
                # steps first used by pass p: those with floor(32k/S) == p
                while steps_issued < 16 and 32 * steps_issued * NPASS < NJ * (p + 1):
                    _issue_step(steps_issued)
                    steps_issued += 1
                nj = S * (p + 1)
                acc = acc_psum.tile([128, S], f32)
                for r in range(nj):
                    cp = 4 * S * p + 3 * S + 511 - 4 * r
                    nc.tensor.matmul(
                        acc[:, :],
                        exp_sb[:, r, :],
                        xp_sb[:, cp : cp + S],
                        start=(r == 0),
                        stop=(r == nj - 1),
                    )
                # psum -> sbuf on DVE, interleaved with expansion in issue
                # order so the in-order queue releases psum promptly (bias
                # is added on the host)
                nc.vector.tensor_copy(out_sb[:, S * p : S * (p + 1)], acc[:, :])
                nc.sync.dma_start(
                    out_d[:, S * p : S * (p + 1)], out_sb[:, S * p : S * (p + 1)]
                )
            while steps_issued < 16:
                _issue_step(steps_issued)
                steps_issued += 1
    _split_excess_waits(nc)
    return nc


def _build_nc_fp8():
    """Split-precision fp8e4m3 DoubleRow kernel (S=32, K=256 per matmul).

    out = W8*X8 + W8*X8r + W8r*X8 accumulated in one psum; the dropped
    W8r*X8r term is ~0.4%% relative, under the bf16 scheme's own error.
    8-tap blocks: block8(j8, q) = 267 - j8 - 4q, k-tile kt covers taps
    4kt..4kt+3 within the block; moving slice offset cp = 128p + 603 - 8*j8
    with the kt shift (+4 cols) folded into a 3D rhs AP.
    """
    import concourse.bass as bass
    import concourse.mybir as mybir
    from concourse.tile import TileContext
    from concourse.ap import AP

    _patch_tile_drain()
    f32 = mybir.dt.float32
    f8 = mybir.dt.float8e4
    u16 = mybir.dt.uint16
    bf16 = mybir.dt.bfloat16
    DR = mybir.MatmulPerfMode.DoubleRow

    SS = 32
    NP8 = 16          # passes
    NJ8 = 256         # 8-tap blocks per full sweep

    nc = bass.Bass()
    xp8_d = nc.declare_dram_parameter("xp8", [128, XPW], f8, isOutput=False)
    xp8r_d = nc.declare_dram_parameter("xp8r", [128, XPW], f8, isOutput=False)
    wc8_d = nc.declare_dram_parameter("wc8", [128, 512 * 32], f8, isOutput=False)
    wc8r_d = nc.declare_dram_parameter("wc8r", [128, 512 * 32], f8, isOutput=False)
    out_d = nc.declare_dram_parameter("out", [128, NP8 * SS], f32, isOutput=True)

    max_passes = int(os.environ.get("CK_MAX_PASSES", str(NP8)))
    n_warmup = int(os.environ.get("CK_WARMUP", "240"))

    with TileContext(nc) as tc:
        with (
            tc.tile_pool(name="const", bufs=1) as const,
            tc.tile_pool(name="work", bufs=1) as work,
            tc.tile_pool(name="acc_psum", bufs=7, space="PSUM") as acc_psum,
            tc.tile_pool(name="warm_psum", bufs=1, space="PSUM") as warm_psum,
        ):
            xp8_sb = const.tile([128, XPW], f8)
            xp8r_sb = const.tile([128, XPW], f8)
            wc8_sb = const.tile([128, 512, 32], f8)
            wc8r_sb = const.tile([128, 512, 32], f8)
            exp8_sb = const.tile([128, NJ8, 2, 128], f8)
            exp8r_sb = const.tile([128, NJ8, 2, 128], f8)
            out_sb = work.tile([128, NP8 * SS], f32)

            def _wc_piece(dst, par, lo, hi):
                nc.sync.dma_start(
                    dst[:, lo:hi, :],
                    par[:, lo * 32 : hi * 32].rearrange("p (a b) -> p a b", b=32),
                )

            def _xp_piece(dst, par, lo, hi):
                nc.sync.dma_start(dst[:, lo:hi], par[:, lo:hi])

            # first-use order; step k needs wc m < 64(k+1) of both tables
            if os.environ.get("CK_DMA8", "L1") == "L1":
                _wc_piece(wc8_sb, wc8_d, 0, 64)
                _wc_piece(wc8r_sb, wc8r_d, 0, 64)
                _xp_piece(xp8_sb, xp8_d, 480, 896)
                _xp_piece(xp8r_sb, xp8r_d, 480, 896)
                _wc_piece(wc8_sb, wc8_d, 64, 192)
                _wc_piece(wc8r_sb, wc8r_d, 64, 192)
                _xp_piece(xp8_sb, xp8_d, 896, 1664)
                _xp_piece(xp8r_sb, xp8r_d, 896, 1664)
                _wc_piece(wc8_sb, wc8_d, 192, 320)
                _wc_piece(wc8r_sb, wc8r_d, 192, 320)
                _xp_piece(xp8_sb, xp8_d, 1664, XPW)
                _xp_piece(xp8r_sb, xp8r_d, 1664, XPW)
                _wc_piece(wc8_sb, wc8_d, 320, 448)
                _wc_piece(wc8r_sb, wc8r_d, 320, 448)
                _wc_piece(wc8_sb, wc8_d, 448, 512)
                _wc_piece(wc8r_sb, wc8r_d, 448, 512)
            else:
                _xp_piece(xp8_sb, xp8_d, 480, 896)
                _xp_piece(xp8r_sb, xp8r_d, 480, 896)
                for lo, hi in ((0, 64), (64, 128), (128, 192)):
                    _wc_piece(wc8_sb, wc8_d, lo, hi)
                    _wc_piece(wc8r_sb, wc8r_d, lo, hi)
                _xp_piece(xp8_sb, xp8_d, 896, 1664)
                _xp_piece(xp8r_sb, xp8r_d, 896, 1664)
                for lo, hi in ((192, 256), (256, 320)):
                    _wc_piece(wc8_sb, wc8_d, lo, hi)
                    _wc_piece(wc8r_sb, wc8r_d, lo, hi)
                _xp_piece(xp8_sb, xp8_d, 1664, XPW)
                _xp_piece(xp8r_sb, xp8r_d, 1664, XPW)
                for lo, hi in ((320, 384), (384, 448), (448, 512)):
                    _wc_piece(wc8_sb, wc8_d, lo, hi)
                    _wc_piece(wc8r_sb, wc8r_d, lo, hi)

            if n_warmup:
                scratch = const.tile([128, 16], bf16)
                nc.vector.memset(scratch[:, :], 1.0)
                warm_ps = warm_psum.tile([16, 16], f32)
                for _ in range(n_warmup):
                    nc.tensor.matmul(
                        warm_ps[:, :], scratch[:, :], scratch[:, :],
                        start=True, stop=True,
                    )

            # zero corners (blocks >= 256): j8 < 12 - 4q
            zlen8 = [12, 8, 4, 0]
            for exp in (exp8_sb, exp8r_sb):
                for q in range(3):
                    nc.gpsimd.memset(
                        exp[:, 0 : zlen8[q], :, 32 * q : 32 * q + 32], 0.0
                    )

            def _copy_chunk(engine, exp, wc, j0, j1, q):
                lo = max(j0, zlen8[q])
                if lo >= j1:
                    return
                m0 = 2 * lo + 8 * q - 24
                dst = exp[:, lo:j1, :, 32 * q : 32 * q + 32]
                srcw = wc[:, m0 : m0 + 2 * (j1 - lo), :].rearrange(
                    "p (a b) c -> p a b c", b=2
                )
                if engine == "act":
                    nc.scalar.activation(
                        dst, srcw, mybir.ActivationFunctionType.Copy, bias=0.0
                    )
                elif engine == "pool":
                    nc.gpsimd.tensor_copy(dst.bitcast(u16), srcw.bitcast(u16))
                else:
                    nc.vector.tensor_copy(dst.bitcast(u16), srcw.bitcast(u16))

            from concourse.ap import AP as _AP

            def _fused_q123(exp, wc, j0, j1):
                # dst free [j8, kt, (q,col)=96] contiguous; src 4-dim
                # overlapping windows m = 2*j8 + kt + 8*q - 24, q=1..3
                base = wc[:, :, :]
                srcw = _AP(
                    base.tensor,
                    base.offset + (2 * j0 - 16) * 32,
                    [list(base.ap[0]), [64, j1 - j0], [32, 2], [256, 3], [1, 32]],
                )
                dst = exp[:, j0:j1, :, 32:128].rearrange(
                    "p a b (c d) -> p a b c d", c=3
                )
                nc.vector.tensor_copy(dst.bitcast(u16), srcw.bitcast(u16))

            def _issue_step(k):
                if k == 0:
                    mode = os.environ.get("CK_STEP0", "full")
                    halves = ((0, 16), (16, 32)) if mode == "half" else ((0, 32),)
                    for jl, jh in halves:
                        for q in (3, 2, 1, 0):
                            _copy_chunk("dve", exp8_sb, wc8_sb, jl, jh, q)
                            _copy_chunk("dve", exp8r_sb, wc8r_sb, jl, jh, q)
                    return
                j0, j1 = 32 * k, 32 * k + 32
                if os.environ.get("CK_FUSE8", "1") == "1":
                    _fused_q123(exp8_sb, wc8_sb, j0, j1)
                    _fused_q123(exp8r_sb, wc8r_sb, j0, j1)
                else:
                    for q in (1, 2, 3):
                        _copy_chunk("dve", exp8_sb, wc8_sb, j0, j1, q)
                        _copy_chunk("dve", exp8r_sb, wc8r_sb, j0, j1, q)
                _copy_chunk("pool", exp8_sb, wc8_sb, j0, j1, 0)
                _copy_chunk("act", exp8r_sb, wc8r_sb, j0, j1, 0)

            xp8_base = xp8_sb[:, :]
            xp8r_base = xp8r_sb[:, :]

            def _rhs(base, cp):
                return AP(
                    base.tensor, base.offset + cp,
                    [list(base.ap[0]), [4, 2], [1, SS]],
                )

            steps_issued = 0
            for p in range(max_passes):
                while steps_issued < 8 and 32 * steps_issued < 16 * (p + 1):
                    _issue_step(steps_issued)
                    steps_issued += 1
                nj = 16 * (p + 1)
                # last pass: two column-chains so the first half's psum-copy
                # + DMA chain overlaps the second half's matmuls
                halves = (
                    ((0, 16), (16, 32))
                    if p == max_passes - 1
                    and os.environ.get("CK_SPLIT_LAST", "0") == "1"
                    else ((0, SS),)
                )
                for c0, c1 in halves:
                    cw = c1 - c0
                    acc = acc_psum.tile([128, cw], f32)

                    def _rhs_h(base, cp):
                        return AP(
                            base.tensor, base.offset + cp + c0,
                            [list(base.ap[0]), [4, 2], [1, cw]],
                        )

                    for r in range(nj):
                        cp = 128 * p + 603 - 8 * r
                        nc.tensor.matmul(
                            acc[:, :], exp8_sb[:, r, :, :], _rhs_h(xp8_base, cp),
                            start=(r == 0), stop=False, perf_mode=DR,
                        )
                        nc.tensor.matmul(
                            acc[:, :], exp8_sb[:, r, :, :], _rhs_h(xp8r_base, cp),
                            start=False, stop=False, perf_mode=DR,
                        )
                        nc.tensor.matmul(
                            acc[:, :], exp8r_sb[:, r, :, :], _rhs_h(xp8_base, cp),
                            start=False, stop=(r == nj - 1), perf_mode=DR,
                        )
                    lo = SS * p + c0
                    nc.vector.tensor_copy(out_sb[:, lo : lo + cw], acc[:, :])
                    nc.sync.dma_start(out_d[:, lo : lo + cw], out_sb[:, lo : lo + cw])
            while steps_issued < 8:
                _issue_step(steps_issued)
                steps_issued += 1
    _split_excess_waits(nc)
    return nc


# ------------------------------------------------------------------- entry

def kernel(**inputs):
    import ml_dtypes
    from concourse.bass_utils import run_bass_kernel_spmd

    x = np.asarray(inputs["x"], dtype=np.float32)
    k = _siren_kernel(
        np.asarray(inputs["pos_rel"]), np.asarray(inputs["w1"]),
        np.asarray(inputs["b1"]), np.asarray(inputs["w2"]),
        np.asarray(inputs["b2"]), np.asarray(inputs["w3"]),
        np.asarray(inputs["b3"]),
    )
    bias = np.asarray(inputs["bias"], dtype=np.float32).reshape(C_OUT, 1)
    impl = os.environ.get("CK_IMPL", "fp8")

    if impl == "fp8":
        WC8, WC8r = _build_fp8_tables(k)
        XP8, XP8r = _build_xp8(x)
        if _cache.get("impl") != impl or "nc" not in _cache:
            _cache["nc"] = _build_nc_fp8()
            _cache["impl"] = impl
        per_core = [
            {"xp8": XP8[b % B], "xp8r": XP8r[b % B], "wc8": WC8, "wc8r": WC8r}
            for b in range(B)
        ]
    else:
        WCR = _build_wcr(k).astype(ml_dtypes.bfloat16)
        XP = _build_xp(x).astype(ml_dtypes.bfloat16)
        if _cache.get("impl") != impl or "nc" not in _cache:
            _cache["nc"] = _build_nc()
            _cache["impl"] = impl
        per_core = [{"xp": XP[b % B], "wcr": WCR} for b in range(B)]
    nc = _cache["nc"]

    n_cores = int(os.environ.get("CK_CORES", str(N_CORES)))
    in_maps = [per_core[b % B] for b in range(n_cores)]

    g_np, g_s = (16, 32) if impl == "fp8" else (NPASS, S)

    def _gather(raw):
        # raw [128, 512]: psum group q holds times 4*s*p + s*q + c
        o4 = raw.reshape(4, 32, g_np, g_s)             # [q, o, p, c]
        return o4.transpose(1, 2, 0, 3).reshape(C_OUT, T) + bias

    # The axon-tunneled device occasionally throws a transient
    # NRT_EXEC_UNIT_UNRECOVERABLE on 8-core launches; retry, then fall back
    # to two 4-core waves (same NEFF, batches split across waves).
    res = None
    for attempt in range(3):
        try:
            res = run_bass_kernel_spmd(nc, in_maps, core_ids=list(range(n_cores)))
            break
        except Exception:
            if attempt == 2:
                res = None
            else:
                continue
    if res is not None:
        out = np.stack(
            [_gather(res.results[b % n_cores]["out"]) for b in range(B)], axis=0
        )
        return out.astype(np.float32)

    half = n_cores // 2 if n_cores > 1 else 1
    outs = []
    for w0 in range(0, B, half):
        wave_maps = [per_core[(w0 + c) % B] for c in range(half)]
        wres = run_bass_kernel_spmd(nc, wave_maps, core_ids=list(range(half)))
        outs.extend(_gather(wres.results[c]["out"]) for c in range(half))
    out = np.stack(outs[:B], axis=0)
    return out.astype(np.float32)


# revision 39
# speedup vs baseline: 1.0185x; 1.0185x over previous
"""CKConv (nn_CKConv_85950885527678) Trainium2 Bass kernel.

Strategy: data-parallel over batch (8 batches -> 8 NeuronCores). The tiny
SIREN kernel network runs on the host; the generated conv kernel is
replicated to every core (per the sharding hint). The bias add also runs
on the host (it would otherwise sit on the device critical path).

Per core the causal conv out[o,t] = sum_{i,l>=1} K[o,i,l] * xp[i,t+l]
(xp = x left-padded with T zeros) is computed with full-width 128x128
matmuls via a *diagonal* decomposition: the causal boundary t+l >= 2048 is
a diagonal in (time, tap) space, so four time-tiles spaced S apart share
one moving-operand slice, each receiving a tap block shifted by S taps.
Every matmul therefore uses all 128 stationary columns (4 tiles x 32 out
channels) and a full-depth contraction.

Default implementation (CK_IMPL=fp8): split-precision float8_e4m3 with
DoubleRow perf mode (K = 256 = 8 taps x 32 in channels, two 4-tap k-tiles
packed per partition; the k-tile's +4-tap shift is folded into a 3D moving
AP with a 4-column offset). out = W8*X8 + W8*X8r + W8r*X8 accumulates in
one f32 psum; the dropped W8r*X8r term is ~0.4% relative, giving overall
~0.002 rel err (better than plain bf16 at ~0.004). A bf16 fallback
(CK_IMPL=bf16, K=128) is kept.

The stationary tables ship compact (each tap block once) and are expanded
4x on-chip by the otherwise-idle DVE/Act/Pool engines (bit-preserving
uint16-bitcast copies, overlapping-window APs), overlapped with PE
compute; a short warmup keeps the PE p-state ramped while the first DMA
pieces land.
"""

import os
import numpy as np

B, C_IN, C_OUT, T, D = 8, 32, 32, 2048, 32
L = T + 1
U0 = 1534
XPW = 2565
S = int(os.environ.get("CK_S", "32"))   # time-tile size
NPASS = T // (4 * S)
NJ = 512                # stationary col-blocks
NBR = 512               # compact table blocks (real, b descending)
N_CORES = 8

_cache = {}


# ---------------------------------------------------------------- host prep

def _siren_kernel(pos_rel, w1, b1, w2, b2, w3, b3):
    p = pos_rel.reshape(1, L).astype(np.float32)
    h = np.sin(w1.astype(np.float32) @ p + b1[:, None].astype(np.float32))
    h = np.sin(w2.astype(np.float32) @ h + b2[:, None].astype(np.float32))
    k = w3.astype(np.float32) @ h + b3[:, None].astype(np.float32)
    return k.astype(np.float32)


def _build_wcr(k):
    """Compact reversed table (real blocks only): col (br-96)*32 + o holds
    K[o, i, 4b+1+dd] at partition dd*32+i, with b = 607-br, br in [96,608)."""
    kk = k.reshape(C_OUT, C_IN, L)[:, :, 1:]
    arr = kk.reshape(C_OUT, C_IN, NJ, 4).transpose(3, 1, 2, 0)  # [dd,i,b,o]
    w = arr.reshape(128, NJ, C_OUT)[:, ::-1, :]                 # b descending
    return np.ascontiguousarray(w.reshape(128, NJ * C_OUT)).astype(np.float32)


def _build_xp(x):
    xpad = np.zeros((B, C_IN, 2 * T + 8), np.float32)
    xpad[:, :, T : 2 * T] = x
    XP = np.empty((B, 128, XPW), np.float32)
    for dd in range(4):
        XP[:, dd * 32 : (dd + 1) * 32, :] = xpad[:, :, U0 + dd : U0 + dd + XPW]
    return XP


def _build_fp8_tables(k):
    """Split-precision fp8 tables.

    Compact stationary layout (kt pairs adjacent, 8-tap blocks descending):
    wc[dd*32+i, m, o] = W[o, i, 4*block4(m) + dd] with
    block4(m) = 510 - m + 2*(m & 1)  (m in [0, 512)).
    """
    import ml_dtypes

    F8 = ml_dtypes.float8_e4m3fn
    kk = k.reshape(C_OUT, C_IN, L)[:, :, 1:]          # [o, i, tap 0..2047]
    w8 = kk.astype(F8)
    w8r = (kk - w8.astype(np.float32)).astype(F8)

    m = np.arange(512)
    block4 = 510 - m + 2 * (m & 1)
    tap0 = 4 * block4

    def compact(w):
        # -> [dd*32+i, m, o]
        taps = w.reshape(C_OUT, C_IN, 512, 4)          # [o, i, b, dd]
        arr = taps[:, :, block4, :]                     # [o, i, m, dd]
        return np.ascontiguousarray(
            arr.transpose(3, 1, 2, 0).reshape(128, 512 * C_OUT)
        )

    c8, c8r = compact(w8), compact(w8r)
    pair = np.stack(
        [c8.reshape(128, 512, C_OUT), c8r.reshape(128, 512, C_OUT)], axis=2
    )
    return np.ascontiguousarray(pair.reshape(128, 512 * 2 * C_OUT))


def _build_xp8(x):
    import ml_dtypes

    F8 = ml_dtypes.float8_e4m3fn
    xpad = np.zeros((B, C_IN, 2 * T + 8), np.float32)
    xpad[:, :, T : 2 * T] = x
    x8 = xpad.astype(F8)
    x8r = (xpad - x8.astype(np.float32)).astype(F8)
    XP8 = np.empty((B, 128, XPW), F8)
    XP8r = np.empty((B, 128, XPW), F8)
    for dd in range(4):
        XP8[:, dd * 32 : (dd + 1) * 32, :] = x8[:, :, U0 + dd : U0 + dd + XPW]
        XP8r[:, dd * 32 : (dd + 1) * 32, :] = x8r[:, :, U0 + dd : U0 + dd + XPW]
    return XP8, XP8r


# ------------------------------------------------------- tile drain patch

def _patch_tile_drain():
    """This walrus build rejects >2 sync waits on a CTRL (Drain) instruction;
    spread the TileContext exit waits over single-wait NOPs instead."""
    from concourse.tile import TileContext
    from concourse.vector_clock import ScopedClock, VectorClock

    if getattr(TileContext, "_ck_drain_patched", False):
        return

    def _drain_and_barrier(self, tick_clock, wait_clock):
        light = os.environ.get("CK_LIGHT_DRAIN", "2")
        gc = tick_clock.global_clock
        n = len(gc)
        if light != "2":
            for p in range(n):
                if gc[p] <= 0:
                    continue
                vec = [gc[q] if q == p else 0 for q in range(n)]
                nop = self.nc.sync.nop(nofuse=True, hint=f"split_drain_wait_p{p}")
                wait_clock.add_sem_waits(
                    nop.ins, ScopedClock({None: VectorClock(vec)})
                )
        self.nc.sync.drain()
        if light != "2":
            self.nc.all_engine_barrier()
        assert self.sems is not None
        popped = self.nc._tile_sem_poison_stack.pop()
        assert popped is self._sem_poison
        if os.environ.get("CK_LIGHT_DRAIN", "2") == "0":
            self.nc.clear_and_free_semaphores(list(self.sems.allocated().values()))
            self.nc.all_engine_barrier()
        else:
            pass  # leave sems allocated; program ends here anyway

    TileContext._drain_and_barrier = _drain_and_barrier
    TileContext._ck_drain_patched = True


WAIT_LIMIT = 1  # this walrus build encodes at most 2 sync waits per instruction


def _split_excess_waits(nc, limit=WAIT_LIMIT):
    """Hoist excess sem waits onto same-engine NOPs placed just before the
    instruction — in-order engine queues make this semantically identical."""
    import concourse.mybir as mybir

    n_split = 0
    for f in nc.m.functions:
        for bb in f.blocks:
            new_insts = []
            changed = False
            for inst in bb.instructions:
                si = inst.sync_info
                waits = list(si.on_wait) if si is not None and si.on_wait else []
                if len(waits) > limit:
                    extra, keep = waits[:-limit], waits[-limit:]
                    for i in range(0, len(extra), limit):
                        n_split += 1
                        new_insts.append(
                            mybir.InstNoOp(
                                name=f"I-ckwsplit-{n_split}",
                                engine=inst.engine,
                                ins=[],
                                outs=[],
                                sync_info=mybir.SyncInfo(
                                    on_wait=extra[i : i + limit], on_update=[]
                                ),
                            )
                        )
                    inst.sync_info = mybir.SyncInfo(
                        on_wait=keep, on_update=list(si.on_update) if si.on_update else []
                    )
                    changed = True
                new_insts.append(inst)
            if changed:
                bb.instructions = new_insts
    return n_split


# ------------------------------------------------------------ device kernel

def _build_nc():
    import concourse.bass as bass
    import concourse.mybir as mybir
    from concourse.tile import TileContext

    _patch_tile_drain()
    f32 = mybir.dt.float32
    bf16 = mybir.dt.bfloat16

    nc = bass.Bass()
    xp_d = nc.declare_dram_parameter("xp", [128, XPW], bf16, isOutput=False)
    wcr_d = nc.declare_dram_parameter("wcr", [128, NJ * 32], bf16, isOutput=False)
    out_d = nc.declare_dram_parameter("out", [128, NPASS * S], f32, isOutput=True)

    max_passes = int(os.environ.get("CK_MAX_PASSES", str(NPASS)))
    n_warmup = int(os.environ.get("CK_WARMUP", "240"))

    with TileContext(nc) as tc:
        with (
            tc.tile_pool(name="const", bufs=1) as const,
            tc.tile_pool(name="work", bufs=1) as work,
            tc.tile_pool(name="acc_psum", bufs=7, space="PSUM") as acc_psum,
            tc.tile_pool(name="warm_psum", bufs=1, space="PSUM") as warm_psum,
        ):
            xp_sb = const.tile([128, XPW], bf16)
            wcr_sb = const.tile([128, NBR, 32], bf16)
            exp_sb = const.tile([128, NJ, 128], bf16)
            out_sb = work.tile([128, NPASS * S], f32)

            # DMA order = first-use order. Pass 0 only needs xp cols
            # >= 515-S and the first 32 compact blocks; xp cols [0, 384)
            # are never read.
            def _wcr_piece(lo, hi):
                nc.sync.dma_start(
                    wcr_sb[:, lo:hi, :],
                    wcr_d[:, lo * 32 : hi * 32].rearrange(
                        "p (a b) -> p a b", b=32
                    ),
                )

            def _xp_piece(lo, hi):
                nc.sync.dma_start(xp_sb[:, lo:hi], xp_d[:, lo:hi])

            xp_layout = os.environ.get("CK_XP", "V3")
            if xp_layout == "V2":
                _wcr_piece(0, 32)
                _xp_piece(480, XPW)
                _wcr_piece(32, 64)
                _wcr_piece(64, 96)
                for k in range(3, 16):
                    _wcr_piece(32 * k, 32 * k + 32)
            else:
                _wcr_piece(0, 32)
                _xp_piece(480, 896)
                _wcr_piece(32, 64)
                _wcr_piece(64, 96)
                _xp_piece(896, 1664)
                _wcr_piece(96, 128)
                _wcr_piece(128, 160)
                _xp_piece(1664, XPW)
                for k in range(5, 16):
                    _wcr_piece(32 * k, 32 * k + 32)

            # p-state warmup: keep the PE busy with trivial matmuls while
            # the first stationary chunks arrive, so real matmuls start at
            # full clock. Feed from a memset scratch so warmup needs no DMA.
            if n_warmup:
                scratch = const.tile([128, 16], bf16)
                nc.vector.memset(scratch[:, :], 1.0)
                warm_ps = warm_psum.tile([16, 16], f32)
                for _ in range(n_warmup):
                    nc.tensor.matmul(
                        warm_ps[:, :], scratch[:, :], scratch[:, :],
                        start=True, stop=True,
                    )

            # On-chip 4x expansion: exp[:, j, 32q:+32] =
            # wcr[:, j + (S/4)q - 3S/4, :] (zero where the block index is
            # out of range, i.e. j < 3S/4 - (S/4)q), issued in 16 steps of
            # 32 j each, each step emitted just before the first pass that
            # reads it. q=0..2 on DVE, q=3 on the Activation engine.
            zlen = [max(0, 3 * S // 4 - (S // 4) * q) for q in range(4)]
            for q in range(4):
                if zlen[q]:
                    # zero corners on the idle Pool engine, off DVE's path
                    nc.gpsimd.memset(
                        exp_sb[:, 0 : zlen[q], 32 * q : 32 * q + 32], 0.0
                    )

            from concourse.ap import AP

            def _issue_step(kstep):
                j0 = 32 * kstep
                if j0 >= zlen[0]:
                    # q1..q3 in one fused DVE copy (overlapping-window src
                    # AP [part, j, q, col]); q0 alternates Act/Pool
                    base = wcr_sb[:, :, :]
                    srcw = AP(
                        base.tensor,
                        base.offset + (j0 - (S // 2)) * 32,
                        [list(base.ap[0]), [32, 32], [(S // 4) * 32, 3], [1, 32]],
                    )
                    dst = exp_sb[:, j0 : j0 + 32, 32:128].rearrange(
                        "p a (b c) -> p a b c", b=3
                    )
                    nc.vector.tensor_copy(dst, srcw)
                    q0src = wcr_sb[:, j0 - 3 * S // 4 : j0 + 32 - 3 * S // 4, :]
                    q0dst = exp_sb[:, j0 : j0 + 32, 0:32]
                    if kstep % 2 == 0:
                        nc.scalar.activation(
                            q0dst, q0src,
                            mybir.ActivationFunctionType.Copy, bias=0.0,
                        )
                    else:
                        nc.gpsimd.tensor_copy(q0dst, q0src)
                else:
                    # first step: all chunks on DVE (small, zero-clipped) so
                    # the slower engines stay off the startup critical path
                    for q in range(3, -1, -1):
                        lo = max(j0, zlen[q])       # first non-zero j
                        if lo >= j0 + 32:
                            continue
                        src_lo = lo + (S // 4) * q - 3 * S // 4
                        nc.vector.tensor_copy(
                            exp_sb[:, lo : j0 + 32, 32 * q : 32 * q + 32],
                            wcr_sb[:, src_lo : src_lo + (j0 + 32 - lo), :],
                        )

            steps_issued = 0
            for p in range(max_passes):
                # steps first used by pass p: those with floor(32k/S) == p
                while steps_issued < 16 and 32 * steps_issued * NPASS < NJ * (p + 1):
                    _issue_step(steps_issued)
                    steps_issued += 1
                nj = S * (p + 1)
                acc = acc_psum.tile([128, S], f32)
                for r in range(nj):
                    cp = 4 * S * p + 3 * S + 511 - 4 * r
                    nc.tensor.matmul(
                        acc[:, :],
                        exp_sb[:, r, :],
                        xp_sb[:, cp : cp + S],
                        start=(r == 0),
                        stop=(r == nj - 1),
                    )
                # psum -> sbuf on DVE, interleaved with expansion in issue
                # order so the in-order queue releases psum promptly (bias
                # is added on the host)
                nc.vector.tensor_copy(out_sb[:, S * p : S * (p + 1)], acc[:, :])
                nc.sync.dma_start(
                    out_d[:, S * p : S * (p + 1)], out_sb[:, S * p : S * (p + 1)]
                )
            while steps_issued < 16:
                _issue_step(steps_issued)
                steps_issued += 1
    _split_excess_waits(nc)
    return nc


def _build_nc_fp8():
    """Split-precision fp8e4m3 DoubleRow kernel (S=32, K=256 per matmul).

    out = W8*X8 + W8*X8r + W8r*X8 accumulated in one psum; the dropped
    W8r*X8r term is ~0.4%% relative, under the bf16 scheme's own error.
    8-tap blocks: block8(j8, q) = 267 - j8 - 4q, k-tile kt covers taps
    4kt..4kt+3 within the block; moving slice offset cp = 128p + 603 - 8*j8
    with the kt shift (+4 cols) folded into a 3D rhs AP.
    """
    import concourse.bass as bass
    import concourse.mybir as mybir
    from concourse.tile import TileContext
    from concourse.ap import AP

    _patch_tile_drain()
    f32 = mybir.dt.float32
    f8 = mybir.dt.float8e4
    u16 = mybir.dt.uint16
    bf16 = mybir.dt.bfloat16
    DR = mybir.MatmulPerfMode.DoubleRow

    SS = 32
    NP8 = 16          # passes
    NJ8 = 256         # 8-tap blocks per full sweep

    nc = bass.Bass()
    xp8_d = nc.declare_dram_parameter("xp8", [128, XPW], f8, isOutput=False)
    xp8r_d = nc.declare_dram_parameter("xp8r", [128, XPW], f8, isOutput=False)
    wcp_d = nc.declare_dram_parameter("wcp", [128, 512 * 64], f8, isOutput=False)
    out_d = nc.declare_dram_parameter("out", [128, NP8 * SS], f32, isOutput=True)

    max_passes = int(os.environ.get("CK_MAX_PASSES", str(NP8)))
    n_warmup = int(os.environ.get("CK_WARMUP", "240"))

    with TileContext(nc) as tc:
        with (
            tc.tile_pool(name="const", bufs=1) as const,
            tc.tile_pool(name="work", bufs=1) as work,
            tc.tile_pool(name="acc_psum", bufs=7, space="PSUM") as acc_psum,
            tc.tile_pool(name="warm_psum", bufs=1, space="PSUM") as warm_psum,
        ):
            xp8_sb = const.tile([128, XPW], f8)
            xp8r_sb = const.tile([128, XPW], f8)
            wcp_sb = const.tile([128, 512, 2, 32], f8)
            exp8_sb = const.tile([128, NJ8, 2, 128], f8)
            exp8r_sb = const.tile([128, NJ8, 2, 128], f8)
            out_sb = work.tile([128, NP8 * SS], f32)

            def _wc_piece(lo, hi):
                nc.sync.dma_start(
                    wcp_sb[:, lo:hi, :, :],
                    wcp_d[:, lo * 64 : hi * 64].rearrange(
                        "p (a b c) -> p a b c", b=2, c=32
                    ),
                )

            def _xp_piece(dst, par, lo, hi):
                nc.sync.dma_start(dst[:, lo:hi], par[:, lo:hi])

            # first-use order; step k needs wc m < 64(k+1) of both tables
            _wc_piece(0, 64)
            _xp_piece(xp8_sb, xp8_d, 480, 896)
            _xp_piece(xp8r_sb, xp8r_d, 480, 896)
            _wc_piece(64, 128)
            _xp_piece(xp8_sb, xp8_d, 896, 1664)
            _wc_piece(128, 192)
            _xp_piece(xp8r_sb, xp8r_d, 896, 1664)
            _wc_piece(192, 320)
            _xp_piece(xp8_sb, xp8_d, 1664, XPW)
            _xp_piece(xp8r_sb, xp8r_d, 1664, XPW)
            _wc_piece(320, 448)
            _wc_piece(448, 512)

            if n_warmup:
                scratch = const.tile([128, 16], bf16)
                nc.vector.memset(scratch[:, :], 1.0)
                warm_ps = warm_psum.tile([16, 16], f32)
                for _ in range(n_warmup):
                    nc.tensor.matmul(
                        warm_ps[:, :], scratch[:, :], scratch[:, :],
                        start=True, stop=True,
                    )

            # zero corners (blocks >= 256): j8 < 12 - 4q
            zlen8 = [12, 8, 4, 0]
            for exp in (exp8_sb, exp8r_sb):
                for q in range(3):
                    nc.gpsimd.memset(
                        exp[:, 0 : zlen8[q], :, 32 * q : 32 * q + 32], 0.0
                    )

            def _copy_chunk(engine, exp, tbl, j0, j1, q):
                lo = max(j0, zlen8[q])
                if lo >= j1:
                    return
                m0 = 2 * lo + 8 * q - 24
                dst = exp[:, lo:j1, :, 32 * q : 32 * q + 32]
                srcw = wcp_sb[:, m0 : m0 + 2 * (j1 - lo), tbl, :].rearrange(
                    "p (a b) c -> p a b c", b=2
                )
                if engine == "act":
                    nc.scalar.activation(
                        dst, srcw, mybir.ActivationFunctionType.Copy, bias=0.0
                    )
                elif engine == "pool":
                    nc.gpsimd.tensor_copy(dst.bitcast(u16), srcw.bitcast(u16))
                else:
                    nc.vector.tensor_copy(dst.bitcast(u16), srcw.bitcast(u16))

            from concourse.ap import AP as _AP

            def _fused_q123(exp, tbl, j0, j1):
                # dst free [j8, kt, (q,col)=96] contiguous; src 4-dim
                # overlapping windows m = 2*j8 + kt + 8*q - 24, q=1..3,
                # strides doubled by the (W8, W8r) pair interleave
                base = wcp_sb[:, :, :, :]
                srcw = _AP(
                    base.tensor,
                    base.offset + (2 * j0 - 16) * 64 + tbl * 32,
                    [list(base.ap[0]), [128, j1 - j0], [64, 2], [512, 3], [1, 32]],
                )
                dst = exp[:, j0:j1, :, 32:128].rearrange(
                    "p a b (c d) -> p a b c d", c=3
                )
                nc.vector.tensor_copy(dst.bitcast(u16), srcw.bitcast(u16))

            def _issue_step(k):
                if k == 0:
                    mode = os.environ.get("CK_STEP0", "full")
                    halves = ((0, 16), (16, 32)) if mode == "half" else ((0, 32),)
                    for jl, jh in halves:
                        for q in (3, 2, 1, 0):
                            _copy_chunk("dve", exp8_sb, 0, jl, jh, q)
                            _copy_chunk("dve", exp8r_sb, 1, jl, jh, q)
                    return
                j0, j1 = 32 * k, 32 * k + 32
                _fused_q123(exp8_sb, 0, j0, j1)
                _fused_q123(exp8r_sb, 1, j0, j1)
                _copy_chunk("pool", exp8_sb, 0, j0, j1, 0)
                _copy_chunk("act", exp8r_sb, 1, j0, j1, 0)

            xp8_base = xp8_sb[:, :]
            xp8r_base = xp8r_sb[:, :]

            def _rhs(base, cp):
                return AP(
                    base.tensor, base.offset + cp,
                    [list(base.ap[0]), [4, 2], [1, SS]],
                )

            steps_issued = 0
            for p in range(max_passes):
                while steps_issued < 8 and 32 * steps_issued < 16 * (p + 1):
                    _issue_step(steps_issued)
                    steps_issued += 1
                nj = 16 * (p + 1)
                # last pass: two column-chains so the first half's psum-copy
                # + DMA chain overlaps the second half's matmuls
                halves = (
                    ((0, 16), (16, 32))
                    if p == max_passes - 1
                    and os.environ.get("CK_SPLIT_LAST", "0") == "1"
                    else ((0, SS),)
                )
                sweep_mode = os.environ.get("CK_SWEEP", "1") == "1"
                for c0, c1 in halves:
                    cw = c1 - c0
                    acc = acc_psum.tile([128, cw], f32)

                    def _rhs_h(base, cp):
                        return AP(
                            base.tensor, base.offset + cp + c0,
                            [list(base.ap[0]), [4, 2], [1, cw]],
                        )

                    if sweep_mode:
                        # three sweeps: the first two touch only the W8
                        # table / X8 first, deferring the wc8r/xp8r chains
                        terms = [
                            (exp8_sb, xp8_base), (exp8_sb, xp8r_base),
                            (exp8r_sb, xp8_base),
                        ]
                        for ti, (stat, mov) in enumerate(terms):
                            for r in range(nj):
                                cp = 128 * p + 603 - 8 * r
                                nc.tensor.matmul(
                                    acc[:, :], stat[:, r, :, :], _rhs_h(mov, cp),
                                    start=(ti == 0 and r == 0),
                                    stop=(ti == 2 and r == nj - 1),
                                    perf_mode=DR,
                                )
                    else:
                        for r in range(nj):
                            cp = 128 * p + 603 - 8 * r
                            nc.tensor.matmul(
                                acc[:, :], exp8_sb[:, r, :, :], _rhs_h(xp8_base, cp),
                                start=(r == 0), stop=False, perf_mode=DR,
                            )
                            nc.tensor.matmul(
                                acc[:, :], exp8_sb[:, r, :, :], _rhs_h(xp8r_base, cp),
                                start=False, stop=False, perf_mode=DR,
                            )
                            nc.tensor.matmul(
                                acc[:, :], exp8r_sb[:, r, :, :], _rhs_h(xp8_base, cp),
                                start=False, stop=(r == nj - 1), perf_mode=DR,
                            )
                    lo = SS * p + c0
                    nc.vector.tensor_copy(out_sb[:, lo : lo + cw], acc[:, :])
                    nc.sync.dma_start(out_d[:, lo : lo + cw], out_sb[:, lo : lo + cw])
            while steps_issued < 8:
                _issue_step(steps_issued)
                steps_issued += 1
    _split_excess_waits(nc)
    return nc


# ------------------------------------------------------------------- entry

def kernel(**inputs):
    import ml_dtypes
    from concourse.bass_utils import run_bass_kernel_spmd

    x = np.asarray(inputs["x"], dtype=np.float32)
    k = _siren_kernel(
        np.asarray(inputs["pos_rel"]), np.asarray(inputs["w1"]),
        np.asarray(inputs["b1"]), np.asarray(inputs["w2"]),
        np.asarray(inputs["b2"]), np.asarray(inputs["w3"]),
        np.asarray(inputs["b3"]),
    )
    bias = np.asarray(inputs["bias"], dtype=np.float32).reshape(C_OUT, 1)
    impl = os.environ.get("CK_IMPL", "fp8")

    if impl == "fp8":
        WCP = _build_fp8_tables(k)
        XP8, XP8r = _build_xp8(x)
        if _cache.get("impl") != impl or "nc" not in _cache:
            _cache["nc"] = _build_nc_fp8()
            _cache["impl"] = impl
        per_core = [
            {"xp8": XP8[b % B], "xp8r": XP8r[b % B], "wcp": WCP}
            for b in range(B)
        ]
    else:
        WCR = _build_wcr(k).astype(ml_dtypes.bfloat16)
        XP = _build_xp(x).astype(ml_dtypes.bfloat16)
        if _cache.get("impl") != impl or "nc" not in _cache:
            _cache["nc"] = _build_nc()
            _cache["impl"] = impl
        per_core = [{"xp": XP[b % B], "wcr": WCR} for b in range(B)]
    nc = _cache["nc"]

    n_cores = int(os.environ.get("CK_CORES", str(N_CORES)))
    in_maps = [per_core[b % B] for b in range(n_cores)]

    g_np, g_s = (16, 32) if impl == "fp8" else (NPASS, S)

    def _gather(raw):
        # raw [128, 512]: psum group q holds times 4*s*p + s*q + c
        o4 = raw.reshape(4, 32, g_np, g_s)             # [q, o, p, c]
        return o4.transpose(1, 2, 0, 3).reshape(C_OUT, T) + bias

    # The axon-tunneled device occasionally throws a transient
    # NRT_EXEC_UNIT_UNRECOVERABLE on 8-core launches; retry, then fall back
    # to two 4-core waves (same NEFF, batches split across waves).
    res = None
    for attempt in range(3):
        try:
            res = run_bass_kernel_spmd(nc, in_maps, core_ids=list(range(n_cores)))
            break
        except Exception:
            if attempt == 2:
                res = None
            else:
                continue
    if res is not None:
        out = np.stack(
            [_gather(res.results[b % n_cores]["out"]) for b in range(B)], axis=0
        )
        return out.astype(np.float32)

    half = n_cores // 2 if n_cores > 1 else 1
    outs = []
    for w0 in range(0, B, half):
        wave_maps = [per_core[(w0 + c) % B] for c in range(half)]
        wres = run_bass_kernel_spmd(nc, wave_maps, core_ids=list(range(half)))
        outs.extend(_gather(wres.results[c]["out"]) for c in range(half))
    out = np.stack(outs[:B], axis=0)
    return out.astype(np.float32)
